# revision 1
# baseline (speedup 1.0000x reference)
"""Trainium2 Bass kernel for the GCN message-passing model (8 NeuronCores).

Strategy
--------
- Nodes (and their incoming edges) are sharded by destination across 8 cores
  (12500 nodes each).  Self-loops are appended as ordinary edges.
- The symmetric normalization dinv[src]*dinv[dst] is folded into row scalings:
  hws = dinv * (h @ W) is what gets gathered;  out[d] = dinv[d] * sum hws[src].
- Per layer each core computes hws for its own shard and the shards are
  AllGather'ed (bf16) so each core can gather any source row locally.
- Edge aggregation: edges are bucketed per 128-destination block and per
  source quarter (so dma_gather's int16 indices address a <=25000-row window).
  Gathered message tiles [128 edges, 128 feat] are aggregated with a TensorE
  matmul against an on-chip one-hot "segment" matrix built by DVE is_equal,
  accumulating feature-major per-block sums in PSUM, then added into a
  feature-major SBUF accumulator.
- Full-tensor LayerNorm uses per-core partial sums + a tiny AllReduce; the
  normalization is applied lazily before the next matmul.
- Graph mean-pooling reuses the segment-matmul trick on the sorted `batch`
  vector, followed by an AllReduce and a tiny replicated MLP + log_softmax.
"""

import sys

sys.path.insert(0, "/opt/trn_rl_repo")

import numpy as np
import ml_dtypes

import concourse.bass as bass
import concourse.bacc as bacc
import concourse.mybir as mybir
import concourse.tile as tile
from concourse.bass_utils import run_bass_kernel_spmd
from concourse.library_config import mlp as gpsimd_mlp_lib
from concourse.masks import make_identity

NCORES = 8
N_NODES = 100_000
F = 128          # feature/hidden width
NCLS = 10
LAYERS = 3
NGRAPH = 256
EPS = 1e-5
NSH = N_NODES // NCORES          # 12500 nodes per core
NBLK = (NSH + 127) // 128        # 98 blocks of 128 dst rows
LASTW = NSH - (NBLK - 1) * 128   # 84 rows in the last block
NQUART = 4
QSZ = N_NODES // NQUART          # 25000-row int16 windows
CHUNK = 1024                     # idxs per dma_gather (single packet)
TPC = CHUNK // 128               # tiles per chunk
NQUEUE = 4
SLABCH = 16                      # gather chunks per idx slab load

BF16 = mybir.dt.bfloat16
F32 = mybir.dt.float32
I16 = mybir.dt.int16
AOP = mybir.AluOpType
AF = mybir.ActivationFunctionType
BF = ml_dtypes.bfloat16


def _host_preprocess(edge_index):
    """Build per-core edge tiling + gather indices.  Integer work only."""
    src = np.asarray(edge_index[0], dtype=np.int64)
    dst = np.asarray(edge_index[1], dtype=np.int64)
    loop = np.arange(N_NODES, dtype=np.int64)
    src = np.concatenate([src, loop])
    dst = np.concatenate([dst, loop])
    deg = np.bincount(dst, minlength=N_NODES).astype(np.float64)
    dinv = (1.0 / np.sqrt(deg)).astype(np.float32)

    core = dst // NSH
    blk = (dst % NSH) // 128
    slot = (dst % NSH) % 128
    quart = src // QSZ

    # group edges by (core, block, quarter)
    key = (core * NBLK + blk) * NQUART + quart
    order = np.argsort(key, kind="stable")
    key_s = key[order]
    src_s = src[order]
    slot_s = slot[order]
    ngroups = NCORES * NBLK * NQUART
    counts = np.bincount(key_s, minlength=ngroups).reshape(NCORES, NBLK, NQUART)
    starts = np.zeros(ngroups + 1, dtype=np.int64)
    np.cumsum(counts.reshape(-1), out=starts[1:])

    # uniform tile grid: T[b][q] = max over cores of ceil(count/128)
    T = np.maximum((counts + 127) // 128, 1).max(axis=0)  # [NBLK, NQUART]
    # pad each quarter's tile total to a chunk multiple so every dma_gather
    # chunk stays within one quarter window; pads extend the last block
    for q in range(NQUART):
        T[NBLK - 1, q] += (-int(T[:, q].sum())) % TPC
    TT = int(T.sum())
    ecap = TT * 128

    # tile stream order: quarter-major, then block
    tile_block = np.empty(TT, dtype=np.int64)
    tile_quarter = np.empty(TT, dtype=np.int64)
    group_len = np.empty(TT, dtype=np.int64)
    t0 = 0
    for q in range(NQUART):
        for b in range(NBLK):
            n = int(T[b, q])
            tile_block[t0:t0 + n] = b
            tile_quarter[t0:t0 + n] = q
            group_len[t0:t0 + n] = n
            t0 += n
    assert t0 == TT

    # per-core edge arrays in stream order (pad idx=0: a real but harmless
    # gather; pad slot=255 matches no iota column so it contributes zero)
    idx16 = np.zeros((NCORES, ecap), dtype=np.int16)
    slots = np.full((NCORES, ecap), 255, dtype=np.float32)
    for c in range(NCORES):
        pos = 0
        for q in range(NQUART):
            for b in range(NBLK):
                g = (c * NBLK + b) * NQUART + q
                s0, s1 = starts[g], starts[g + 1]
                n = s1 - s0
                idx16[c, pos:pos + n] = (src_s[s0:s1] - q * QSZ).astype(np.int16)
                slots[c, pos:pos + n] = slot_s[s0:s1].astype(np.float32)
                pos += int(T[b, q]) * 128
        assert pos == ecap

    # device layouts: gather idx wrapped [16, n/16] replicated to 128 rows;
    # slot tags [128, TT] with edge j*128+p at [p, j]
    idxw = idx16.reshape(NCORES, -1, 16).transpose(0, 2, 1)
    idxw = np.ascontiguousarray(np.tile(idxw, (1, 8, 1)))
    slotw = np.ascontiguousarray(
        slots.reshape(NCORES, TT, 128).transpose(0, 2, 1)).astype(BF)

    meta = dict(TT=TT, tile_block=tile_block, tile_quarter=tile_quarter,
                group_len=group_len)
    return dinv, idxw, slotw, meta


def _build_program(meta, cut="full"):
    import os
    gsrc_ext = os.environ.get("GSRC", "") == "ext"
    gidx_pre = os.environ.get("GIDX", "") == "pre"
    gq0 = os.environ.get("GQ", "") == "0"
    gn = int(os.environ.get("GN", "0"))
    """Trace the SPMD Bass/Tile program (shared by all 8 cores)."""
    TT = meta["TT"]
    tile_block = meta["tile_block"]
    tile_quarter = meta["tile_quarter"]
    group_len = meta["group_len"]
    ECAP = TT * 128
    NCHUNK = ECAP // CHUNK

    nc = bacc.Bacc("TRN2", target_bir_lowering=False, debug=False,
                   num_devices=NCORES, num_swdge_queues=NQUEUE)

    # ---- external inputs (per core) ----
    xT_in = nc.declare_dram_parameter("xT", [F, NBLK * 128], F32, isOutput=False)
    idx_in = nc.declare_dram_parameter("idx", [128, ECAP // 16], I16, isOutput=False)
    slot_in = nc.declare_dram_parameter("slot", [128, TT], BF16, isOutput=False)
    dinvrep_in = nc.declare_dram_parameter("dinvrep", [128, NBLK * 128], F32, isOutput=False)
    dinvw_in = nc.declare_dram_parameter("dinvw", [128, NBLK], F32, isOutput=False)
    pslot_in = nc.declare_dram_parameter("pslot", [128, NBLK], BF16, isOutput=False)
    iota128_in = nc.declare_dram_parameter("iota128", [128, 128], BF16, isOutput=False)
    iota256_in = nc.declare_dram_parameter("iota256", [128, 256], BF16, isOutput=False)
    lin1W_in = nc.declare_dram_parameter("lin1W", [F, F], F32, isOutput=False)
    lin1b_in = nc.declare_dram_parameter("lin1b", [F, 1], F32, isOutput=False)
    convW_in = nc.declare_dram_parameter("convW", [F, LAYERS * F], BF16, isOutput=False)
    convb_in = nc.declare_dram_parameter("convb", [F, LAYERS], F32, isOutput=False)
    mlpW1_in = nc.declare_dram_parameter("mlpW1", [F, F], BF16, isOutput=False)
    mlpb1_in = nc.declare_dram_parameter("mlpb1", [F, 1], F32, isOutput=False)
    mlpW2_in = nc.declare_dram_parameter("mlpW2", [F, NCLS], BF16, isOutput=False)
    mlpb2r_in = nc.declare_dram_parameter("mlpb2r", [128, NCLS], F32, isOutput=False)
    invcntr_in = nc.declare_dram_parameter("invcntr", [128, NGRAPH], F32, isOutput=False)
    hws_ext_in = (nc.declare_dram_parameter("hws_ext", [NCORES * NSH, F], BF16,
                                            isOutput=False) if gsrc_ext else None)
    out_ext = nc.declare_dram_parameter("out", [NGRAPH, NCLS], F32, isOutput=True)

    rg = [list(range(NCORES))]

    with tile.TileContext(nc) as tc:
        with tc.tile_pool(name="const", bufs=1) as cst, \
             tc.tile_pool(name="big", bufs=1) as big, \
             tc.tile_pool(name="work", bufs=4) as work, \
             tc.tile_pool(name="gbuf", bufs=6) as gpool, \
             tc.tile_pool(name="idxs", bufs=3) as ipool, \
             tc.tile_pool(name="psum", bufs=3, space="PSUM") as pp, \
             tc.tile_pool(name="ppool", bufs=1, space="PSUM") as ppool, \
             tc.tile_pool(name="dram", bufs=1, space="DRAM") as dram, \
             tc.tile_pool(name="dram2", bufs=1, space="DRAM") as dram2:

            nc.gpsimd.load_library(gpsimd_mlp_lib)

            # ---- persistent SBUF constants ----
            slot_t = cst.tile([128, TT], BF16)
            nc.sync.dma_start(out=slot_t[:], in_=slot_in[:])
            dinvrep = cst.tile([128, NBLK * 128], F32)
            nc.sync.dma_start(out=dinvrep[:], in_=dinvrep_in[:])
            dinvw = cst.tile([128, NBLK], F32)
            nc.sync.dma_start(out=dinvw[:], in_=dinvw_in[:])
            pslot = cst.tile([128, NBLK], BF16)
            nc.sync.dma_start(out=pslot[:], in_=pslot_in[:])
            iota128 = cst.tile([128, 128], BF16)
            nc.sync.dma_start(out=iota128[:], in_=iota128_in[:])
            iota256 = cst.tile([128, 256], BF16)
            nc.sync.dma_start(out=iota256[:], in_=iota256_in[:])
            lin1W = cst.tile([F, F], F32)
            nc.sync.dma_start(out=lin1W[:], in_=lin1W_in[:])
            lin1b = cst.tile([F, 1], F32)
            nc.sync.dma_start(out=lin1b[:], in_=lin1b_in[:])
            convW = cst.tile([F, LAYERS * F], BF16)
            nc.sync.dma_start(out=convW[:], in_=convW_in[:])
            convb = cst.tile([F, LAYERS], F32)
            nc.sync.dma_start(out=convb[:], in_=convb_in[:])
            ones_col = cst.tile([128, 1], BF16)
            nc.vector.memset(ones_col[:], 1.0)
            ones_row1 = cst.tile([1, 128], BF16)
            nc.vector.memset(ones_row1[:], 1.0)
            ident = cst.tile([128, 128], BF16)
            make_identity(nc, ident[:])

            # feature-major accumulator for the current layer (also h' store)
            acc = big.tile([128, NBLK * 128], F32)

            # pre-zero the rotating gather buffers (DMA skips pad rows, so
            # whatever bits are there must at least be finite floats)
            for _ in range(6):
                g0 = gpool.tile([128, TPC, F], BF16, tag="gb")
                nc.vector.memset(g0[:].rearrange("p a b -> p (a b)"), 0.0)

            # DRAM: AllGather bounce + full hws table
            hws_shard = dram.tile([NSH, F], BF16)
            hws_full = dram.tile([NCORES * NSH, F], BF16)
            gather_src = hws_ext_in if gsrc_ext else hws_full
            quarter_ap = [gather_src[q * QSZ:(q + 1) * QSZ, :]
                          for q in range(NQUART)]

            # ---------------- helpers ----------------
            def emit_shard_matmul(i, get_lhsT):
                """Per block: psum = lhsT_b.T @ W_i -> *dinv -> hws_shard."""
                W = convW[:, i * F:(i + 1) * F]
                for b in range(NBLK):
                    w = 128 if b < NBLK - 1 else LASTW
                    lhsT_b = get_lhsT(b)
                    ps = pp.tile([128, F], F32, tag="mm")
                    nc.tensor.matmul(out=ps[:w, :], lhsT=lhsT_b[:, :w], rhs=W,
                                     start=True, stop=True)
                    hb = work.tile([128, F], BF16, tag="hws")
                    nc.vector.tensor_scalar(
                        out=hb[:w, :], in0=ps[:w, :],
                        scalar1=dinvw[:w, b:b + 1], scalar2=None,
                        op0=AOP.mult)
                    nc.sync.dma_start(out=hws_shard[b * 128:b * 128 + w, :],
                                      in_=hb[:w, :])

            def emit_allgather():
                nc.gpsimd.collective_compute(
                    "AllGather", AOP.bypass, replica_groups=rg,
                    ins=[hws_shard[:]], outs=[hws_full[:]])

            # ---------------- P0: h1 = relu(x@lin1+b); hws1 ----------------
            def p0_lhsT(b):
                xb = work.tile([128, 128], F32, tag="xb")
                nc.sync.dma_start(out=xb[:],
                                  in_=xT_in[:, b * 128:(b + 1) * 128])
                ps = pp.tile([128, 128], F32, tag="mm")
                nc.tensor.matmul(out=ps[:], lhsT=lin1W[:], rhs=xb[:],
                                 start=True, stop=True)
                h1b = work.tile([128, 128], BF16, tag="h1")
                nc.scalar.activation(out=h1b[:], in_=ps[:], func=AF.Relu,
                                     bias=lin1b[:], scale=1.0)
                return h1b[:]

            if cut != "probe":
                emit_shard_matmul(0, p0_lhsT)
                if cut != "p0" and not gsrc_ext:
                    emit_allgather()
            else:
                probe_acc = work.tile([128, F], F32, tag="pacc")
                nc.vector.memset(probe_acc[:], 0.0)
                idx_all2 = cst.tile([128, ECAP // 16], I16, tag="idxall2")
                nc.sync.dma_start(out=idx_all2[:], in_=idx_in[:])
                nrun = int(os.environ.get("GN", "64"))
                for ch in range(min(nrun, NCHUNK)):
                    qq = int(tile_quarter[ch * TPC])
                    gb = gpool.tile([128, TPC, F], BF16, tag="gb")
                    nc.gpsimd.dma_gather(
                        gb[:], quarter_ap[qq],
                        idx_all2[:, ch * (CHUNK // 16):(ch + 1) * (CHUNK // 16)],
                        CHUNK, CHUNK, F, single_packet=True, queue_num=0)
                    nc.vector.tensor_tensor(
                        out=probe_acc[:, 0:1], in0=probe_acc[:, 0:1],
                        in1=gb[:].rearrange("p a b -> p (a b)")[:, 0:1],
                        op=AOP.add)
            nlayers_run = {"p0": 0, "ag": 0, "probe": 0, "l0g": 1, "l0gs": 1,
                           "l0gm": 1, "l0p1": 1, "l0": 1,
                           "l01": 2}.get(cut, LAYERS)

            # ---------------- conv layers ----------------
            pool_ps = None
            for li in range(nlayers_run):
                # ---- pass 1: edge aggregation into `acc` (feature-major) ----
                for b in range(NBLK):
                    nc.vector.memset(acc[:, b * 128:(b + 1) * 128], 0.0)
                stats = work.tile([128, 2], F32, tag="stats")
                nc.vector.memset(stats[:], 0.0)

                gtile = 0
                open_psum = None
                open_block = -1
                remaining = 0
                idx_slab = None
                if gidx_pre:
                    idx_all = cst.tile([128, ECAP // 16], I16)
                    nc.sync.dma_start(out=idx_all[:], in_=idx_in[:])
                nchunk_run = NCHUNK if gn == 0 else min(gn, NCHUNK)
                for ch in range(nchunk_run):
                    if gidx_pre:
                        pass
                    elif ch % SLABCH == 0:
                        idx_slab = ipool.tile([128, SLABCH * CHUNK // 16], I16,
                                              tag="idxslab")
                        wsl = min(SLABCH * CHUNK, ECAP - ch * CHUNK) // 16
                        nc.sync.dma_start(
                            out=idx_slab[:, :wsl],
                            in_=idx_in[:, ch * CHUNK // 16:
                                       ch * CHUNK // 16 + wsl])
                    qq = int(tile_quarter[gtile])
                    gb = gpool.tile([128, TPC, F], BF16, tag="gb")
                    if gidx_pre:
                        idx_ap = idx_all[:, ch * (CHUNK // 16):(ch + 1) * (CHUNK // 16)]
                    else:
                        off = (ch % SLABCH) * (CHUNK // 16)
                        idx_ap = idx_slab[:, off:off + CHUNK // 16]
                    nc.gpsimd.dma_gather(
                        gb[:], quarter_ap[qq], idx_ap,
                        CHUNK, CHUNK, F, single_packet=True,
                        queue_num=0 if gq0 else ch % NQUEUE)
                    gbf = gb[:].rearrange("p a b -> p (a b)")
                    if cut == "l0g":
                        # consume gb cheaply so gathers aren't dead code
                        nc.vector.tensor_tensor(
                            out=acc[:, 0:1], in0=acc[:, 0:1],
                            in1=gbf[:, 0:1], op=AOP.add)
                        gtile += TPC
                        continue
                    for t in range(TPC):
                        b = int(tile_block[gtile])
                        if b != open_block and cut not in ("l0g", "l0gs"):
                            if open_psum is not None:
                                nc.vector.tensor_tensor(
                                    out=acc[:, open_block * 128:(open_block + 1) * 128],
                                    in0=acc[:, open_block * 128:(open_block + 1) * 128],
                                    in1=open_psum[:], op=AOP.add)
                            open_psum = pp.tile([128, 128], F32, tag="agg")
                            open_block = b
                            remaining = int(group_len[gtile])
                        seg = work.tile([128, 128], BF16, tag="seg")
                        nc.vector.tensor_tensor(
                            out=seg[:],
                            in0=slot_t[:, gtile:gtile + 1].to_broadcast([128, 128]),
                            in1=iota128[:], op=AOP.is_equal)
                        if cut == "l0gs":
                            nc.vector.tensor_tensor(
                                out=acc[:, 0:1], in0=acc[:, 0:1],
                                in1=seg[:, 0:1], op=AOP.add)
                            nc.vector.tensor_tensor(
                                out=acc[:, 1:2], in0=acc[:, 1:2],
                                in1=gbf[:, t * F:t * F + 1], op=AOP.add)
                        else:
                            nc.tensor.matmul(
                                out=open_psum[:], lhsT=gbf[:, t * F:(t + 1) * F],
                                rhs=(iota128[:] if cut == "l0gm" else seg[:]),
                                start=(remaining == int(group_len[gtile])),
                                stop=(remaining == 1))
                        remaining -= 1
                        gtile += 1
                if cut not in ("l0g", "l0gs"):
                    nc.vector.tensor_tensor(
                        out=acc[:, open_block * 128:(open_block + 1) * 128],
                        in0=acc[:, open_block * 128:(open_block + 1) * 128],
                        in1=open_psum[:], op=AOP.add)
                open_psum = None
                open_block = -1

                if cut in ("l0g", "l0gs", "l0gm"):
                    break
                # ---- epilogue: h' = relu(dinv*acc + bias); LN partials ----
                for b in range(NBLK):
                    w = 128 if b < NBLK - 1 else LASTW
                    ab = acc[:, b * 128:b * 128 + w]
                    nc.vector.tensor_tensor(
                        out=ab, in0=ab,
                        in1=dinvrep[:, b * 128:b * 128 + w], op=AOP.mult)
                    s1 = work.tile([128, 1], F32, tag="s1")
                    nc.scalar.activation(out=ab, in_=ab, func=AF.Relu,
                                         bias=convb[:, li:li + 1], scale=1.0,
                                         accum_out=s1[:])
                    sq = work.tile([128, 128], F32, tag="sq")
                    s2 = work.tile([128, 1], F32, tag="s2")
                    nc.scalar.activation(out=sq[:, :w], in_=ab, func=AF.Square,
                                         bias=0.0, scale=1.0, accum_out=s2[:])
                    nc.vector.tensor_tensor(out=stats[:, 0:1], in0=stats[:, 0:1],
                                            in1=s1[:], op=AOP.add)
                    nc.vector.tensor_tensor(out=stats[:, 1:2], in0=stats[:, 1:2],
                                            in1=s2[:], op=AOP.add)

                if cut == "l0p1":
                    break
                # ---- LayerNorm stats all-reduce + scalars ----
                st_in = dram2.tile([128, 2], F32, tag="stin")
                st_out = dram2.tile([128, 2], F32, tag="stout")
                nc.sync.dma_start(out=st_in[:], in_=stats[:])
                nc.gpsimd.collective_compute(
                    "AllReduce", AOP.add, replica_groups=rg,
                    ins=[st_in[:]], outs=[st_out[:]])
                stg = work.tile([128, 2], F32, tag="stg")
                nc.sync.dma_start(out=stg[:], in_=st_out[:])
                stg16 = work.tile([128, 2], BF16, tag="stg16")
                nc.vector.tensor_copy(out=stg16[:], in_=stg[:])
                ps_s = pp.tile([1, 2], F32, tag="mm")
                nc.tensor.matmul(out=ps_s[:], lhsT=ones_col[:], rhs=stg16[:],
                                 start=True, stop=True)
                sc = work.tile([1, 4], F32, tag="sc")
                nc.scalar.activation(out=sc[:, 0:2], in_=ps_s[:], func=AF.Copy,
                                     bias=0.0, scale=1.0 / (N_NODES * F))
                nc.vector.tensor_tensor(out=sc[:, 2:3], in0=sc[:, 0:1],
                                        in1=sc[:, 0:1], op=AOP.mult)
                nc.vector.tensor_tensor(out=sc[:, 2:3], in0=sc[:, 1:2],
                                        in1=sc[:, 2:3], op=AOP.subtract)
                nc.vector.tensor_scalar(out=sc[:, 2:3], in0=sc[:, 2:3],
                                        scalar1=EPS, scalar2=None,
                                        op0=AOP.add)
                nc.vector.reciprocal(out=sc[:, 3:4], in_=sc[:, 2:3])
                nc.scalar.activation(out=sc[:, 3:4], in_=sc[:, 3:4],
                                     func=AF.Sqrt, bias=0.0, scale=1.0)
                sc16 = work.tile([1, 4], BF16, tag="sc16")
                nc.vector.tensor_copy(out=sc16[:], in_=sc[:])
                ps_b = pp.tile([128, 4], F32, tag="mm")
                nc.tensor.matmul(out=ps_b[:], lhsT=ones_row1[:], rhs=sc16[:],
                                 start=True, stop=True)
                musd = work.tile([128, 4], F32, tag="musd")
                nc.vector.tensor_copy(out=musd[:], in_=ps_b[:])

                # ---- pass 2 ----
                if li < LAYERS - 1:
                    def get_lhsT(b, _m=musd):
                        w = 128 if b < NBLK - 1 else LASTW
                        hb = work.tile([128, 128], BF16, tag="hnorm")
                        nc.vector.tensor_scalar(
                            out=hb[:, :w], in0=acc[:, b * 128:b * 128 + w],
                            scalar1=_m[:, 0:1], scalar2=_m[:, 3:4],
                            op0=AOP.subtract, op1=AOP.mult)
                        return hb[:]
                    emit_shard_matmul(li + 1, get_lhsT)
                    emit_allgather()
                else:
                    # last layer: LN, transpose to node-major, pool per graph
                    pool_ps = ppool.tile([128, NGRAPH], F32, tag="pool")
                    for b in range(NBLK):
                        w = 128 if b < NBLK - 1 else LASTW
                        hb = work.tile([128, 128], BF16, tag="hnorm")
                        nc.vector.tensor_scalar(
                            out=hb[:, :w], in0=acc[:, b * 128:b * 128 + w],
                            scalar1=musd[:, 0:1], scalar2=musd[:, 3:4],
                            op0=AOP.subtract, op1=AOP.mult)
                        ps_t = pp.tile([128, 128], BF16, tag="mm")
                        nc.tensor.transpose(out=ps_t[:], in_=hb[:],
                                            identity=ident[:])
                        h3 = work.tile([128, 128], BF16, tag="h3")
                        nc.vector.tensor_copy(out=h3[:w, :], in_=ps_t[:w, :])
                        segp = work.tile([128, NGRAPH], BF16, tag="segp")
                        nc.vector.tensor_tensor(
                            out=segp[:w, :],
                            in0=pslot[:w, b:b + 1].to_broadcast([w, NGRAPH]),
                            in1=iota256[:w, :], op=AOP.is_equal)
                        nc.tensor.matmul(out=pool_ps[:], lhsT=h3[:w, :],
                                         rhs=segp[:w, :],
                                         start=(b == 0), stop=(b == NBLK - 1),
                                         skip_group_check=True)

            # ---------------- pooled AllReduce + MLP head ----------------
            if pool_ps is None:
                z = work.tile([128, NCLS], F32, tag="zz")
                nc.vector.memset(z[:], 0.0)
                nc.sync.dma_start(out=out_ext[0:128, :], in_=z[:])
                nc.sync.dma_start(out=out_ext[128:256, :], in_=z[:])
            else:
                pooledT = work.tile([128, NGRAPH], F32, tag="pooledT")
                nc.vector.tensor_copy(out=pooledT[:], in_=pool_ps[:])
                pl_in = dram2.tile([128, NGRAPH], F32, tag="plin")
                pl_out = dram2.tile([128, NGRAPH], F32, tag="plout")
                nc.sync.dma_start(out=pl_in[:], in_=pooledT[:])
                nc.gpsimd.collective_compute(
                    "AllReduce", AOP.add, replica_groups=rg,
                    ins=[pl_in[:]], outs=[pl_out[:]])
                pooled = work.tile([128, NGRAPH], F32, tag="pooled2")
                nc.sync.dma_start(out=pooled[:], in_=pl_out[:])
                invcnt = work.tile([128, NGRAPH], F32, tag="invcnt")
                nc.sync.dma_start(out=invcnt[:], in_=invcntr_in[:])
                nc.vector.tensor_tensor(out=pooled[:], in0=pooled[:],
                                        in1=invcnt[:], op=AOP.mult)
                pooled16 = work.tile([128, NGRAPH], BF16, tag="pooled16")
                nc.vector.tensor_copy(out=pooled16[:], in_=pooled[:])

                mlpW1 = work.tile([F, F], BF16, tag="mlpW1")
                nc.sync.dma_start(out=mlpW1[:], in_=mlpW1_in[:])
                mlpb1 = work.tile([F, 1], F32, tag="mlpb1")
                nc.sync.dma_start(out=mlpb1[:], in_=mlpb1_in[:])
                mlpW2 = work.tile([F, NCLS], BF16, tag="mlpW2")
                nc.sync.dma_start(out=mlpW2[:], in_=mlpW2_in[:])
                mlpb2r = work.tile([128, NCLS], F32, tag="mlpb2r")
                nc.sync.dma_start(out=mlpb2r[:], in_=mlpb2r_in[:])

                ps_g = pp.tile([128, NGRAPH], F32, tag="mm")
                nc.tensor.matmul(out=ps_g[:], lhsT=mlpW1[:], rhs=pooled16[:],
                                 start=True, stop=True)
                gT = work.tile([128, NGRAPH], BF16, tag="gT")
                nc.scalar.activation(out=gT[:], in_=ps_g[:], func=AF.Relu,
                                     bias=mlpb1[:], scale=1.0)
                for half in range(2):
                    ps_sc = pp.tile([128, NCLS], F32, tag="mm")
                    nc.tensor.matmul(out=ps_sc[:],
                                     lhsT=gT[:, half * 128:(half + 1) * 128],
                                     rhs=mlpW2[:], start=True, stop=True)
                    scr = work.tile([128, NCLS], F32, tag="scr")
                    nc.vector.tensor_tensor(out=scr[:], in0=ps_sc[:],
                                            in1=mlpb2r[:], op=AOP.add)
                    mx = work.tile([128, 1], F32, tag="mx")
                    nc.vector.tensor_reduce(out=mx[:], in_=scr[:],
                                            axis=mybir.AxisListType.X,
                                            op=AOP.max)
                    nc.vector.tensor_scalar(out=scr[:], in0=scr[:], scalar1=mx[:],
                                            scalar2=None, op0=AOP.subtract)
                    ex = work.tile([128, NCLS], F32, tag="ex")
                    sm = work.tile([128, 1], F32, tag="sm")
                    nc.scalar.activation(out=ex[:], in_=scr[:], func=AF.Exp,
                                         bias=0.0, scale=1.0, accum_out=sm[:])
                    ls = work.tile([128, 1], F32, tag="ls")
                    nc.scalar.activation(out=ls[:], in_=sm[:], func=AF.Ln,
                                         bias=0.0, scale=1.0)
                    nc.vector.tensor_scalar(out=scr[:], in0=scr[:], scalar1=ls[:],
                                            scalar2=None, op0=AOP.subtract)
                    nc.sync.dma_start(out=out_ext[half * 128:(half + 1) * 128, :],
                                      in_=scr[:])

    nc.compile()
    return nc


def _wrap_cols(vec, fill):
    """[NSH] -> [128, NBLK] with node b*128+p at [p, b]."""
    padded = np.full(NBLK * 128, fill, np.float32)
    padded[:NSH] = vec
    return np.ascontiguousarray(padded.reshape(NBLK, 128).T)


def _prepare(inputs):
    x = np.asarray(inputs["x"], dtype=np.float32)
    edge_index = np.asarray(inputs["edge_index"])
    batch = np.asarray(inputs["batch"], dtype=np.int64)
    assert x.shape == (N_NODES, F), x.shape

    dinv, idxw, slotw, meta = _host_preprocess(edge_index)

    cnt = np.bincount(batch, minlength=NGRAPH).astype(np.float64)
    invcnt = (1.0 / np.maximum(cnt, 1.0)).astype(np.float32)
    iota128 = np.broadcast_to(np.arange(128, dtype=np.float32), (128, 128))
    iota256 = np.broadcast_to(np.arange(256, dtype=np.float32), (128, 256))

    lin1_W = np.asarray(inputs["lin1_W"], np.float32)
    lin1_b = np.asarray(inputs["lin1_b"], np.float32)
    conv_W = np.asarray(inputs["conv_W"], np.float32)
    conv_b = np.asarray(inputs["conv_b"], np.float32)
    mlp_W1 = np.asarray(inputs["mlp_W1"], np.float32)
    mlp_b1 = np.asarray(inputs["mlp_b1"], np.float32)
    mlp_W2 = np.asarray(inputs["mlp_W2"], np.float32)
    mlp_b2 = np.asarray(inputs["mlp_b2"], np.float32)

    convW_cat = np.concatenate([conv_W[l] for l in range(LAYERS)], axis=1)

    in_maps = []
    for c in range(NCORES):
        lo, hi = c * NSH, (c + 1) * NSH
        xT = np.zeros((F, NBLK * 128), np.float32)
        xT[:, :NSH] = x[lo:hi].T
        dinv_pad = np.zeros(NBLK * 128, np.float32)
        dinv_pad[:NSH] = dinv[lo:hi]
        in_maps.append({
            "xT": xT,
            "idx": idxw[c],
            "slot": slotw[c],
            "dinvrep": np.ascontiguousarray(
                np.broadcast_to(dinv_pad, (128, NBLK * 128))),
            "dinvw": _wrap_cols(dinv[lo:hi], 0.0),
            "pslot": _wrap_cols(batch[lo:hi].astype(np.float32),
                                300.0).astype(BF),
            "iota128": iota128.astype(BF),
            "iota256": iota256.astype(BF),
            "lin1W": lin1_W,
            "lin1b": np.ascontiguousarray(lin1_b.reshape(F, 1)),
            "convW": convW_cat.astype(BF),
            "convb": np.ascontiguousarray(conv_b.T),
            "mlpW1": mlp_W1.astype(BF),
            "mlpb1": np.ascontiguousarray(mlp_b1.reshape(F, 1)),
            "mlpW2": mlp_W2.astype(BF),
            "mlpb2r": np.ascontiguousarray(
                np.broadcast_to(mlp_b2, (128, NCLS)).astype(np.float32)),
            "invcntr": np.ascontiguousarray(
                np.broadcast_to(invcnt, (128, NGRAPH))),
        })
    return meta, in_maps


_CACHED = {}


def kernel_run(inputs, trace=False):
    import os
    meta, in_maps = _prepare(inputs)
    if os.environ.get("GSRC", "") == "ext":
        z = np.zeros((NCORES * NSH, F), BF)
        for m in in_maps:
            m["hws_ext"] = z
    cut = os.environ.get("KCUT", "full")
    key = (meta["TT"], cut, os.environ.get("GSRC", ""), os.environ.get("GIDX", ""),
           os.environ.get("GQ", ""), os.environ.get("GN", ""))
    if key not in _CACHED:
        _CACHED[key] = _build_program(meta, cut=cut)
    nc = _CACHED[key]
    res = run_bass_kernel_spmd(nc, in_maps, core_ids=list(range(NCORES)),
                               trace=trace)
    out = np.asarray(res.results[0]["out"], dtype=np.float32)
    return out, res.exec_time_ns


def kernel(**inputs):
    out, _ = kernel_run(inputs, trace=False)
    return out

